# revision 9
# baseline (speedup 1.0000x reference)
"""Trainium2 Bass kernel for nn_AttentionMaskGenerator.

Math (verified against the reference):
  z[b,s,t] = x[b,s,:] @ W[t,:] + bias[t] - ln(-ln(u[b,s,t] + 1e-10) + 1e-10)
  tt[b,s]  = argmax_t z  (softmax + straight-through chain == plain argmax)
  row s of the [S,S] mask is:
    - tt == 1 : only the diagonal element (forward window ∩ causal == diag)
    - else    : full causal prefix  (next_global > s always, so the "local"
                constraint never binds under causal)
  output = broadcast over the 8 KV heads -> [B, 8, S, S] f32.

Sharding: data-parallel over (batch x head-pairs). Core c handles batch
c // 4 and emits that batch's [S,S] mask twice (head pair 2*(c%4), 2*(c%4)+1).
Each core writes only the non-zero (lower-trapezoid) columns of each
128-row block; ExternalOutput DRAM is pre-zeroed by the runtime (both the
native run_neff path and the bass2jax/PJRT path), so the strictly-upper
triangle is never written.
"""

import numpy as np

B, S, D, T, H = 2, 2048, 128, 3, 8
P = 128           # partitions / row-block size
NB = S // P       # 16 row blocks
EPS = 1e-10
N_CORES = 8

_CACHE = {}


def _build_program():
    import concourse.bass as bass
    import concourse.bacc as bacc
    import concourse.tile as tile
    from concourse import mybir
    from contextlib import ExitStack

    f32 = mybir.dt.float32
    Alu = mybir.AluOpType
    Act = mybir.ActivationFunctionType

    nc = bacc.Bacc("TRN2", debug=False, num_devices=N_CORES)
    x_ap = nc.dram_tensor("x", [S, D], f32, kind="ExternalInput").ap()
    u_ap = nc.dram_tensor("u", [S, T], f32, kind="ExternalInput").ap()
    w_ap = nc.dram_tensor("w", [T, D], f32, kind="ExternalInput").ap()
    bb_ap = nc.dram_tensor("bb", [T], f32, kind="ExternalInput").ap()
    out_ap = nc.dram_tensor("out", [2, S, S], f32, kind="ExternalOutput").ap()

    with tile.TileContext(nc) as tc, ExitStack() as ctx:
        singles = ctx.enter_context(tc.tile_pool(name="singles", bufs=1))
        outp = ctx.enter_context(tc.tile_pool(name="outp", bufs=6))

        # --- constants / full-input loads (gpsimd queue; sync queue is for stores) ---
        zeros = singles.tile([P, S], f32)
        nc.vector.memset(zeros, 0.0)
        ones128 = singles.tile([P, P], f32)
        nc.gpsimd.memset(ones128, 1.0)
        # diag128[p, j] = 1.0 if j == p else 0.0
        diag128 = singles.tile([P, P], f32)
        nc.gpsimd.affine_select(
            out=diag128, in_=ones128, pattern=[[-1, P]],
            compare_op=Alu.is_equal, fill=0.0, base=0, channel_multiplier=1,
        )
        # W broadcast to every partition: wb[p, t, d] = W[t, d]
        wb = singles.tile([P, T, D], f32)
        nc.gpsimd.dma_start(out=wb, in_=bass.AP(w_ap.tensor, 0, [[0, P], [D, T], [1, D]]))
        # bias broadcast: bbb[p, t] = bias[t]
        bbb = singles.tile([P, T], f32)
        nc.gpsimd.dma_start(out=bbb, in_=bass.AP(bb_ap.tensor, 0, [[0, P], [1, T]]))
        eps_t = singles.tile([P, 1], f32)
        nc.vector.memset(eps_t, EPS)

        # x_all[p, i, d] = x[128*i + p, d]   (whole batch slice, one DMA)
        x_all = singles.tile([P, NB, D], f32)
        nc.gpsimd.dma_start(
            out=x_all, in_=bass.AP(x_ap.tensor, 0, [[D, P], [P * D, NB], [1, D]])
        )
        # u_all[p, i, t] = u[128*i + p, t]
        u_all = singles.tile([P, NB, T], f32)
        nc.gpsimd.dma_start(
            out=u_all, in_=bass.AP(u_ap.tensor, 0, [[T, P], [P * T, NB], [1, T]])
        )

        nf = singles.tile([P, NB], f32)
        nfa = nf[:]

        chunks = ctx.enter_context(tc.tile_pool(name="chunks", bufs=2))
        CB = 4  # row blocks per phase-A chunk
        ba = bbb[:]

        for c0 in range(0, NB, CB):
            # --- phase A for blocks [c0, c0+CB) ---
            # prod[p, i, t, d] = x_all[p, c0+i, d] * wb[p, t, d]
            prod = chunks.tile([P, CB, T, D], f32)
            xa = x_all[:, c0 : c0 + CB, :]
            x_b = bass.AP(xa.tensor, xa.offset, [xa.ap[0], xa.ap[1], [0, T], xa.ap[2]])
            wa = wb[:]
            w_b = bass.AP(wa.tensor, wa.offset, [wa.ap[0], [0, CB], wa.ap[1], wa.ap[2]])
            nc.vector.tensor_mul(prod[:], x_b, w_b)
            # logits[p, i, t] = sum_d prod[p, i, t, d]
            logits = chunks.tile([P, CB, T], f32)
            nc.vector.reduce_sum(logits[:], prod[:], axis=mybir.AxisListType.X)

            # gumbel: g = -ln(-ln(u + eps) + eps); z = logits + g + bias
            g1 = chunks.tile([P, CB, T], f32)
            nc.scalar.activation(
                g1[:], u_all[:, c0 : c0 + CB, :], Act.Ln, bias=eps_t[:, 0:1], scale=1.0
            )
            g2 = chunks.tile([P, CB, T], f32)
            nc.scalar.activation(g2[:], g1[:], Act.Ln, bias=eps_t[:, 0:1], scale=-1.0)

            z = chunks.tile([P, CB, T], f32)
            nc.vector.tensor_sub(z[:], logits[:], g2[:])
            b_b = bass.AP(ba.tensor, ba.offset, [ba.ap[0], [0, CB], ba.ap[1]])
            nc.vector.tensor_add(z[:], z[:], b_b)

            # notflag[p, i] = (z1 <= z0) | (z1 < z2)  == !(argmax picks index 1)
            za = z[:]

            def zcol(t):
                return bass.AP(za.tensor, za.offset + t, [za.ap[0], za.ap[1]])

            c1 = chunks.tile([P, CB], f32)
            nc.vector.tensor_tensor(c1[:], zcol(1), zcol(0), op=Alu.is_le)
            c2 = chunks.tile([P, CB], f32)
            nc.vector.tensor_tensor(c2[:], zcol(1), zcol(2), op=Alu.is_lt)
            nc.vector.tensor_max(nfa[:, c0 : c0 + CB], c1[:], c2[:])

            # --- phase B for blocks [c0, c0+CB) ---
            for i in range(c0, c0 + CB):
                r0 = P * i
                w_cols = r0 + P
                nf_col = nfa[:, i : i + 1]

                ot = outp.tile([P, S], f32)
                if r0 > 0:
                    # prefix cols [0, r0): notflag broadcast (ACT)
                    nc.scalar.activation(
                        ot[:, 0:r0], zeros[:, 0:r0], Act.Identity,
                        bias=nf_col, scale=1.0,
                    )
                # diag chunk: notflag broadcast, trimmed to lower triangle (GPSIMD)
                nf_bcast = bass.AP(nfa.tensor, nfa.offset + i, [nfa.ap[0], [0, P]])
                nc.gpsimd.affine_select(
                    out=ot[:, r0:w_cols], in_=nf_bcast, pattern=[[-1, P]],
                    compare_op=Alu.is_ge, fill=0.0, base=0, channel_multiplier=1,
                )
                # force the diagonal to 1 (covers tt==1 rows) (DVE)
                nc.vector.tensor_max(ot[:, r0:w_cols], ot[:, r0:w_cols], diag128)

                nc.sync.dma_start(
                    out=out_ap[0, r0 : r0 + P, 0:w_cols], in_=ot[:, 0:w_cols]
                )
                nc.sync.dma_start(
                    out=out_ap[1, r0 : r0 + P, 0:w_cols], in_=ot[:, 0:w_cols]
                )

    nc.compile()
    return nc


def _get_program():
    if "nc" not in _CACHE:
        _CACHE["nc"] = _build_program()
    return _CACHE["nc"]


def _make_in_maps(input_tensor, gumbel_u, W, b):
    x = np.ascontiguousarray(np.asarray(input_tensor, dtype=np.float32))
    u = np.ascontiguousarray(np.asarray(gumbel_u, dtype=np.float32))
    w = np.ascontiguousarray(np.asarray(W, dtype=np.float32))
    bb = np.ascontiguousarray(np.asarray(b, dtype=np.float32))
    in_maps = []
    for c in range(N_CORES):
        bi = c // (N_CORES // B)
        in_maps.append({"x": x[bi], "u": u[bi], "w": w, "bb": bb})
    return in_maps


def _assemble(results):
    full = np.empty((B, H, S, S), dtype=np.float32)
    for c in range(N_CORES):
        bi = c // (N_CORES // B)
        q = c % (N_CORES // B)
        full[bi, 2 * q] = results[c]["out"][0]
        full[bi, 2 * q + 1] = results[c]["out"][1]
    return full


def kernel(input_tensor, token_types, gumbel_u, W, b, **_ignored):
    from concourse.bass_utils import run_bass_kernel_spmd

    nc = _get_program()
    in_maps = _make_in_maps(input_tensor, gumbel_u, W, b)
    res = run_bass_kernel_spmd(nc, in_maps, core_ids=list(range(N_CORES)))
    return _assemble(res.results)


# revision 10
# speedup vs baseline: 1.1691x; 1.1691x over previous
"""Trainium2 Bass kernel for nn_AttentionMaskGenerator.

Math (verified against the reference):
  z[b,s,t] = x[b,s,:] @ W[t,:] + bias[t] - ln(-ln(u[b,s,t] + 1e-10) + 1e-10)
  tt[b,s]  = argmax_t z  (softmax + straight-through chain == plain argmax)
  row s of the [S,S] mask is:
    - tt == 1 : only the diagonal element (forward window ∩ causal == diag)
    - else    : full causal prefix  (next_global > s always, so the "local"
                constraint never binds under causal)
  output = broadcast over the 8 KV heads -> [B, 8, S, S] f32.

Sharding: data-parallel over (batch x head-pairs). Core c handles batch
c // 4 and emits that batch's [S,S] mask twice (head pair 2*(c%4), 2*(c%4)+1).
Each core writes only the non-zero (lower-trapezoid) columns of each
128-row block; ExternalOutput DRAM is pre-zeroed by the runtime (both the
native run_neff path and the bass2jax/PJRT path), so the strictly-upper
triangle is never written.

Engine split: PE computes the logits (transpose + [128x128]@[128x3]
matmuls — it is otherwise idle), ACT does the Ln chain and half the
per-row broadcasts, DVE does the other half plus the small compare ops
and the diagonal fix-up, GPSIMD does the triangle trim via affine_select,
and the sync (HWDGE) queue streams the output stores back-to-back.
"""

import numpy as np

B, S, D, T, H = 2, 2048, 128, 3, 8
P = 128           # partitions / row-block size
NB = S // P       # 16 row blocks
CB = 4            # row blocks per phase-A chunk
EPS = 1e-10
N_CORES = 8

_CACHE = {}


def _build_program():
    import concourse.bass as bass
    import concourse.bacc as bacc
    import concourse.tile as tile
    from concourse import mybir
    from contextlib import ExitStack

    f32 = mybir.dt.float32
    Alu = mybir.AluOpType
    Act = mybir.ActivationFunctionType

    nc = bacc.Bacc("TRN2", debug=False, num_devices=N_CORES)
    x_ap = nc.dram_tensor("x", [S, D], f32, kind="ExternalInput").ap()
    u_ap = nc.dram_tensor("u", [S, T], f32, kind="ExternalInput").ap()
    w_ap = nc.dram_tensor("w", [T, D], f32, kind="ExternalInput").ap()
    bb_ap = nc.dram_tensor("bb", [T], f32, kind="ExternalInput").ap()
    out_ap = nc.dram_tensor("out", [2, S, S], f32, kind="ExternalOutput").ap()

    with tile.TileContext(nc) as tc, ExitStack() as ctx:
        singles = ctx.enter_context(tc.tile_pool(name="singles", bufs=1))
        outp = ctx.enter_context(tc.tile_pool(name="outp", bufs=6))
        chunks = ctx.enter_context(tc.tile_pool(name="chunks", bufs=2))
        xts = ctx.enter_context(tc.tile_pool(name="xts", bufs=3))
        psum_t = ctx.enter_context(tc.tile_pool(name="psum_t", bufs=3, space="PSUM"))
        psum_l = ctx.enter_context(tc.tile_pool(name="psum_l", bufs=2, space="PSUM"))

        # --- input loads (sync/HWDGE queue — it is idle until stores begin) ---
        # x_all[p, i, d] = x[128*i + p, d]
        x_all = singles.tile([P, NB, D], f32)
        nc.sync.dma_start(
            out=x_all, in_=bass.AP(x_ap.tensor, 0, [[D, P], [P * D, NB], [1, D]])
        )
        # u_all[p, i, t] = u[128*i + p, t]
        u_all = singles.tile([P, NB, T], f32)
        nc.sync.dma_start(
            out=u_all, in_=bass.AP(u_ap.tensor, 0, [[T, P], [P * T, NB], [1, T]])
        )
        # wt[d, t] = W[t, d]
        wt = singles.tile([P, T], f32)
        nc.sync.dma_start(out=wt, in_=bass.AP(w_ap.tensor, 0, [[1, P], [D, T]]))

        # --- constants ---
        zeros = singles.tile([P, S], f32)
        nc.vector.memset(zeros, 0.0)
        eps_t = singles.tile([P, 1], f32)
        nc.vector.memset(eps_t, EPS)
        ones128 = singles.tile([P, P], f32)
        nc.gpsimd.memset(ones128, 1.0)
        # diag128[p, j] = 1.0 if j == p else 0.0 (identity; also transpose helper)
        diag128 = singles.tile([P, P], f32)
        nc.gpsimd.affine_select(
            out=diag128, in_=ones128, pattern=[[-1, P]],
            compare_op=Alu.is_equal, fill=0.0, base=0, channel_multiplier=1,
        )
        # bias broadcast: bbb[p, t] = bias[t]
        bbb = singles.tile([P, T], f32)
        nc.gpsimd.dma_start(out=bbb, in_=bass.AP(bb_ap.tensor, 0, [[0, P], [1, T]]))

        nf = singles.tile([P, NB], f32)
        nfa = nf[:]
        ba = bbb[:]

        for c0 in range(0, NB, CB):
            # --- phase A for blocks [c0, c0+CB): logits on PE ---
            lg = psum_l.tile([P, CB, T], f32)
            for j in range(CB):
                tp = psum_t.tile([P, P], f32)
                nc.tensor.transpose(tp[:], x_all[:, c0 + j, :], diag128[:])
                xt = xts.tile([P, P], f32)
                nc.vector.tensor_copy(xt[:], tp[:])
                nc.tensor.matmul(lg[:, j, :], lhsT=xt[:], rhs=wt[:], start=True, stop=True)

            # gumbel: g = -ln(-ln(u + eps) + eps); z = logits + g + bias
            g1 = chunks.tile([P, CB, T], f32)
            nc.scalar.activation(
                g1[:], u_all[:, c0 : c0 + CB, :], Act.Ln, bias=eps_t[:, 0:1], scale=1.0
            )
            g2 = chunks.tile([P, CB, T], f32)
            nc.scalar.activation(g2[:], g1[:], Act.Ln, bias=eps_t[:, 0:1], scale=-1.0)

            z = chunks.tile([P, CB, T], f32)
            nc.vector.tensor_sub(z[:], lg[:], g2[:])
            b_b = bass.AP(ba.tensor, ba.offset, [ba.ap[0], [0, CB], ba.ap[1]])
            nc.vector.tensor_add(z[:], z[:], b_b)

            # notflag[p, i] = (z1 <= z0) | (z1 < z2)  == !(argmax picks index 1)
            za = z[:]

            def zcol(t):
                return bass.AP(za.tensor, za.offset + t, [za.ap[0], za.ap[1]])

            c1 = chunks.tile([P, CB], f32)
            nc.vector.tensor_tensor(c1[:], zcol(1), zcol(0), op=Alu.is_le)
            c2 = chunks.tile([P, CB], f32)
            nc.vector.tensor_tensor(c2[:], zcol(1), zcol(2), op=Alu.is_lt)
            nc.vector.tensor_max(nfa[:, c0 : c0 + CB], c1[:], c2[:])

            # --- phase B for blocks [c0, c0+CB) ---
            for i in range(c0, c0 + CB):
                r0 = P * i
                w_cols = r0 + P
                nf_col = nfa[:, i : i + 1]

                ot = outp.tile([P, S], f32)
                if r0 > 0:
                    # prefix cols [0, r0): notflag broadcast (alternate ACT/DVE)
                    if i % 2 == 0:
                        nc.scalar.activation(
                            ot[:, 0:r0], zeros[:, 0:r0], Act.Identity,
                            bias=nf_col, scale=1.0,
                        )
                    else:
                        nc.vector.tensor_scalar_add(ot[:, 0:r0], zeros[:, 0:r0], nf_col)
                # diag chunk: notflag broadcast, trimmed to lower triangle (GPSIMD)
                nf_bcast = bass.AP(nfa.tensor, nfa.offset + i, [nfa.ap[0], [0, P]])
                nc.gpsimd.affine_select(
                    out=ot[:, r0:w_cols], in_=nf_bcast, pattern=[[-1, P]],
                    compare_op=Alu.is_ge, fill=0.0, base=0, channel_multiplier=1,
                )
                # force the diagonal to 1 (covers tt==1 rows) (DVE)
                nc.vector.tensor_max(ot[:, r0:w_cols], ot[:, r0:w_cols], diag128)

                nc.sync.dma_start(
                    out=out_ap[0, r0 : r0 + P, 0:w_cols], in_=ot[:, 0:w_cols]
                )
                nc.sync.dma_start(
                    out=out_ap[1, r0 : r0 + P, 0:w_cols], in_=ot[:, 0:w_cols]
                )

    nc.compile()
    return nc


def _get_program():
    if "nc" not in _CACHE:
        _CACHE["nc"] = _build_program()
    return _CACHE["nc"]


def _make_in_maps(input_tensor, gumbel_u, W, b):
    x = np.ascontiguousarray(np.asarray(input_tensor, dtype=np.float32))
    u = np.ascontiguousarray(np.asarray(gumbel_u, dtype=np.float32))
    w = np.ascontiguousarray(np.asarray(W, dtype=np.float32))
    bb = np.ascontiguousarray(np.asarray(b, dtype=np.float32))
    in_maps = []
    for c in range(N_CORES):
        bi = c // (N_CORES // B)
        in_maps.append({"x": x[bi], "u": u[bi], "w": w, "bb": bb})
    return in_maps


def _assemble(results):
    full = np.empty((B, H, S, S), dtype=np.float32)
    for c in range(N_CORES):
        bi = c // (N_CORES // B)
        q = c % (N_CORES // B)
        full[bi, 2 * q] = results[c]["out"][0]
        full[bi, 2 * q + 1] = results[c]["out"][1]
    return full


def kernel(input_tensor, token_types, gumbel_u, W, b, **_ignored):
    from concourse.bass_utils import run_bass_kernel_spmd

    nc = _get_program()
    in_maps = _make_in_maps(input_tensor, gumbel_u, W, b)
    res = run_bass_kernel_spmd(nc, in_maps, core_ids=list(range(N_CORES)))
    return _assemble(res.results)
